# revision 1
# baseline (speedup 1.0000x reference)
"""Trainium2 Bass kernel: single-head causal self-attention.

Problem: x:(8,2048,1024) f32, Wk/Wq/Wv:(1024,64) f32
  k,q,v = x@Wk, x@Wq, x@Wv ; S = q k^T / sqrt(64) causal-masked
  out = softmax(S) @ v  -> (8,2048,64) f32

Sharding: data-parallel over batch B=8 across the 8 NeuronCores (one batch
element per core).

Per-core design (fp16 matmul paths, fp32 PSUM accumulation):
  - Host passes x^T (C,T) fp16 per core so matmuls contract over C on the
    partition dim with weights stationary.
  - Warm-up: a few dummy matmuls run while x streams in so the PE HAM clock
    gate is already at 2.4 GHz when real work starts.
  - Projections: psum(128,512) = [Wk|Wv]_c^T @ x^T_c accumulated over the 8
    C-tiles -> rows 0:64 = k^T, rows 64:128 = v^T (packed buffer "kvt");
    q^T projected separately (M=64).
  - v^T -> v natural (keys on partitions) via PE transpose against an
    identity block at base-partition 64; a ones-column is appended (V') so
    the PV matmul also produces the softmax denominator.
  - Scores transposed: S^T_j = K_j Q^T (keys on partitions, queries free)
    per 512-query block; causally-dead column ranges are never computed.
  - exp(scale*S^T) on the scalar engine (its only activation -> one table
    load); no row-max subtraction (|scale*S| < ~4). Diagonal 128x128 chunks
    are masked AFTER exp by a multiplicative gpsimd affine_select (p=0).
  - out'^T = V'^T P^T accumulated in PSUM over key tiles; row 64 = sum(P).
  - Epilogue per 512-block: copy out'^T to SBUF (fp16), PE-transpose the
    four (65,128) chunks to natural (128,65), reciprocal of the denominator
    column (8-cycle, per-partition) and broadcast-multiply on DVE. Output is
    written natural (2048,64) fp32, so the host does no transpose.
"""

import os
import sys
from contextlib import ExitStack

import numpy as np

if "/opt/trn_rl_repo" not in sys.path:
    sys.path.insert(0, "/opt/trn_rl_repo")

import concourse.bacc as bacc
import concourse.bass as bass
import concourse.mybir as mybir
import concourse.tile as tile
from concourse.bass import ds
from concourse.bass_utils import run_bass_kernel_spmd
from concourse.masks import make_identity

F32 = mybir.dt.float32
F16 = mybir.dt.float16

B, T, C, H = 8, 2048, 1024, 64
P = 128           # partitions
CT = C // P       # 8 c-tiles
NBLK = 4          # query blocks of 512
QB = T // NBLK    # 512 queries per block
KT = T // P       # 16 key tiles
SCALE = H ** -0.5
N_WARM = 8


def build_bass():
    nc = bacc.Bacc("TRN2")

    xt = nc.dram_tensor("xt", (C, T), F16, kind="ExternalInput")
    wkv = nc.dram_tensor("wkv", (C, 2 * H), F16, kind="ExternalInput")
    wq = nc.dram_tensor("wq", (C, H), F16, kind="ExternalInput")
    out = nc.dram_tensor("out", (T, H), F32, kind="ExternalOutput")
    outv = out.rearrange("(i p) h -> p i h", p=P)   # (128, 16, 64) view

    with ExitStack() as ctx:
        tc = ctx.enter_context(tile.TileContext(nc))
        const = ctx.enter_context(tc.tile_pool(name="const", bufs=1))
        ptp = ctx.enter_context(tc.tile_pool(name="ptp", bufs=4))
        sml = ctx.enter_context(tc.tile_pool(name="sml", bufs=2))
        psA = ctx.enter_context(tc.tile_pool(name="psA", bufs=4, space="PSUM"))
        psO = ctx.enter_context(tc.tile_pool(name="psO", bufs=2, space="PSUM"))
        psT = ctx.enter_context(tc.tile_pool(name="psT", bufs=2, space="PSUM"))

        # ---- persistent SBUF ----
        xt_sb = const.tile([P, CT, T], F16)        # x^T, c-tiled
        wkv_sb = const.tile([P, CT, 2 * H], F16)
        wq_sb = const.tile([P, CT, H], F16)
        kvt = const.tile([P, T], F16)              # rows 0:64 k^T, 64:128 v^T
        qt = const.tile([H, T], F16)               # q^T
        vsb = const.tile([P, KT, H + 1], F16)      # V' tiles (v | ones-col)
        outn = const.tile([P, KT, H], F32)         # natural out tiles
        ident = const.tile([P, P], F16)
        wrm = const.tile([P, QB], F16)             # warm-up operand

        # ---- constants (no DMA deps -> issue immediately) ----
        nc.gpsimd.memset(wrm[:], 0.25)
        make_identity(nc, ident)
        nc.gpsimd.memset(vsb[:, :, H:H + 1], 1.0)  # V' ones-column

        nc.scalar.dma_start(wkv_sb[:], wkv.rearrange("(c p) m -> p c m", p=P))
        nc.scalar.dma_start(wq_sb[:], wq.rearrange("(c p) m -> p c m", p=P))
        for c in range(CT):
            eng = nc.scalar if c % 2 == 0 else nc.gpsimd
            eng.dma_start(xt_sb[:, c, :], xt[ds(c * P, P), :])

        # ---- PE warm-up while x loads: keeps the HAM clock-gate at 2.4 GHz
        warm = []
        for w in range(N_WARM):
            pw = psT.tile([P, QB], F32, tag="tr")
            nc.tensor.matmul(pw[:], wrm[:, 0:P], wrm[:], start=True, stop=True)

        def warm_mm():
            pw = psT.tile([P, QB], F32, tag="tr")
            nc.tensor.matmul(pw[:], wrm[:, 0:P], wrm[:], start=True, stop=True)

        def proj_chunk(g, warm=False):
            sl = ds(g * QB, QB)
            pk = psA.tile([P, QB], F32, tag="mm")
            for c in range(CT):
                nc.tensor.matmul(pk[:], wkv_sb[:, c, :], xt_sb[:, c, sl],
                                 start=(c == 0), stop=(c == CT - 1))
                if warm:  # x still streaming in: keep the PE clock warm
                    warm_mm()
            nc.vector.tensor_copy(kvt[:, sl], pk[:])
            pq = psA.tile([H, QB], F32, tag="mm")
            for c in range(CT):
                nc.tensor.matmul(pq[:], wq_sb[:, c, :], xt_sb[:, c, sl],
                                 start=(c == 0), stop=(c == CT - 1))
                if warm:
                    warm_mm()
            nc.vector.tensor_copy(qt[:, sl], pq[:])

        def v_nat(g):
            # 4 transposed v chunks into one psum, single batched copy out
            pn = psT.tile([P, 4, H], F32, tag="tr")
            for i in range(4):
                t = 4 * g + i
                nc.tensor.matmul(pn[:, i, :], kvt[H:P, ds(t * P, P)],
                                 ident[H:P, H:H + H], start=True, stop=True)
            nc.vector.tensor_copy(vsb[:, ds(4 * g, 4), 0:H], pn[:])

        def make_bg(g):
            # thunks that project chunk g / build its V' tiles; interleaved
            # into the previous attention block so the scalar engine never
            # starves waiting for the next block's scores
            sl = ds(g * QB, QB)
            pk = psA.tile([P, QB], F32, tag="mm")
            pq = psA.tile([H, QB], F32, tag="mm")
            th = []
            for c in range(CT):
                th.append(lambda c=c: nc.tensor.matmul(
                    pk[:], wkv_sb[:, c, :], xt_sb[:, c, sl],
                    start=(c == 0), stop=(c == CT - 1)))
                th.append(lambda c=c: nc.tensor.matmul(
                    pq[:], wq_sb[:, c, :], xt_sb[:, c, sl],
                    start=(c == 0), stop=(c == CT - 1)))
            th.append(lambda: nc.vector.tensor_copy(kvt[:, sl], pk[:]))
            th.append(lambda: nc.vector.tensor_copy(qt[:, sl], pq[:]))
            pn = psT.tile([P, 4, H], F32, tag="tr")
            for i in range(4):
                th.append(lambda i=i: nc.tensor.matmul(
                    pn[:, i, :], kvt[H:P, ds((4 * g + i) * P, P)],
                    ident[H:P, H:H + H], start=True, stop=True))
            th.append(lambda: nc.vector.tensor_copy(vsb[:, ds(4 * g, 4), 0:H],
                                                    pn[:]))
            return th

        def attn_block(b, bg=()):
            po = psO.tile([H + 1, QB], F32, tag="o")
            jmax = 4 * b + 3
            pts = []

            def pv(j):
                pt, c0 = pts[j]
                nc.tensor.matmul(po[:, c0:], vsb[:, j, :], pt[:, c0:],
                                 start=(j == 0), stop=(j == jmax))

            for j in range(jmax + 1):
                c0 = max(0, P * j - QB * b)
                ps = psA.tile([P, QB], F32, tag="mm")
                nc.tensor.matmul(ps[:, c0:], kvt[0:H, ds(j * P, P)],
                                 qt[:, ds(b * QB + c0, QB - c0)],
                                 start=True, stop=True)
                pt = ptp.tile([P, QB], F16, tag="pt")
                nc.scalar.activation(pt[:, c0:], ps[:, c0:],
                                     mybir.ActivationFunctionType.Exp,
                                     scale=SCALE)
                if P * j >= QB * b:  # diagonal: zero p where key s > query u
                    nc.gpsimd.affine_select(
                        out=pt[:, c0:c0 + P], in_=pt[:, c0:c0 + P],
                        compare_op=mybir.AluOpType.is_ge, fill=0.0,
                        base=0, pattern=[[1, P]], channel_multiplier=-1,
                    )
                pts.append((pt, c0))
                if j > 0:
                    pv(j - 1)
                per = -(-len(bg) // (jmax + 1))
                for th in bg[per * j: per * (j + 1)]:
                    th()
            pv(jmax)

            # epilogue: transpose to natural, divide by denominator column
            posb = sml.tile([H + 1, QB], F16, tag="os")
            nc.vector.tensor_copy(posb[:], po[:])
            pn = psT.tile([P, 4, H + 1], F32, tag="tr")
            for i in range(4):
                nc.tensor.matmul(pn[:, i, :], posb[:, ds(i * P, P)],
                                 ident[0:H + 1, 0:H + 1], start=True, stop=True)
            onat = sml.tile([P, 4, H + 1], F32, tag="on")
            nc.vector.tensor_copy(onat[:], pn[:])
            rc = sml.tile([P, 4, 1], F32, tag="rc")
            nc.vector.reciprocal(rc[:], onat[:, :, H:H + 1])
            nc.vector.tensor_tensor(outn[:, ds(4 * b, 4), :],
                                    onat[:, :, 0:H],
                                    rc[:].to_broadcast((P, 4, H)),
                                    mybir.AluOpType.mult)
            nc.scalar.dma_start(outv[:, ds(4 * b, 4), :], outn[:, ds(4 * b, 4), :])

        proj_chunk(0, warm=True)
        v_nat(0)
        for b in range(NBLK):
            bg = make_bg(b + 1) if b + 1 < NBLK else []
            attn_block(b, bg)

    nc.compile()
    return nc


_NC = None
LAST_EXEC_TIME_NS = None  # filled when BASS_TRACE=1 (read by test.py)
LAST_RESULT = None


def _get_nc():
    global _NC
    if _NC is None:
        _NC = build_bass()
    return _NC


def kernel(x, Wk, Wq, Wv):
    global LAST_EXEC_TIME_NS, LAST_RESULT
    x = np.ascontiguousarray(x, dtype=np.float16)
    wkv = np.ascontiguousarray(np.concatenate([Wk, Wv], axis=1), dtype=np.float16)
    wq = np.ascontiguousarray(Wq, dtype=np.float16)

    in_maps = []
    for b in range(B):
        in_maps.append({
            "xt": np.ascontiguousarray(x[b].T),
            "wkv": wkv,
            "wq": wq,
        })

    nc = _get_nc()
    res = run_bass_kernel_spmd(nc, in_maps, list(range(B)))
    LAST_EXEC_TIME_NS = res.exec_time_ns
    LAST_RESULT = res
    out = np.stack([np.ascontiguousarray(m["out"]) for m in res.results])
    return out.astype(np.float32)



# revision 4
# speedup vs baseline: 1.1672x; 1.1672x over previous
"""Trainium2 Bass kernel: single-head causal self-attention.

Problem: x:(8,2048,1024) f32, Wk/Wq/Wv:(1024,64) f32
  k,q,v = x@Wk, x@Wq, x@Wv ; S = q k^T / sqrt(64) causal-masked
  out = softmax(S) @ v  -> (8,2048,64) f32

Sharding: data-parallel over batch B=8 across the 8 NeuronCores (one batch
element per core).

Per-core design (fp16 matmul paths, fp32 PSUM accumulation):
  - x^T (C,T) fp16 streamed chunk-major (512-query chunks) on BOTH HWDGE
    rings (SP + ACT) so projections start ~4us in.
  - Warm-up matmuls on a memset tile un-throttle the PE HAM clock gate
    while x streams.
  - Projections: [Wk|Wq] packed (M=128) -> kqn (k rows 0:64, q rows 64:128);
    DVE cross-base copies build qk2 (q top, k bottom) so scores can be
    ROW-PACKED: two K=64 score matmuls run concurrently in PE halves.
  - V projection: col-packed chunk pairs (two M=64 matmuls in PE column
    halves); v^T -> natural via PE transposes.
  - Scores per (block, tile-pair): 2 concurrent K=64 matmuls -> one fp32
    PSUM group (2 banks); ONE exp ACT call per pair (1024 cols).
  - Diagonal pairs: gpsimd affine_select fills the dead region (everything
    left of/above the causal diagonal) with 0 AFTER exp.
  - PV: po(65,512) += V'_j^T P_j^T accumulated over tiles; row 64 = sum(P)
    (ones-column denominator trick).
  - Epilogue per block: PE-transpose to natural, DVE reciprocal+broadcast
    multiply, natural (2048,64) f32 output, per-block DMA on SP ring.
"""

import sys
from contextlib import ExitStack

import numpy as np

if "/opt/trn_rl_repo" not in sys.path:
    sys.path.insert(0, "/opt/trn_rl_repo")

import concourse.bacc as bacc
import concourse.mybir as mybir
import concourse.tile as tile
from concourse.bass import ds
from concourse.bass_utils import run_bass_kernel_spmd
from concourse.masks import make_identity

F32 = mybir.dt.float32
F16 = mybir.dt.float16

B, T, C, H = 8, 2048, 1024, 64
P = 128           # partitions
CT = C // P       # 8 c-tiles
NBLK = 4          # query blocks of 512
QB = T // NBLK    # 512 queries per block
KT = T // P       # 16 key tiles
SCALE = H ** -0.5
N_WARM = 8


def build_bass():
    nc = bacc.Bacc("TRN2")

    xt = nc.dram_tensor("xt", (C, T), F16, kind="ExternalInput")
    wkq = nc.dram_tensor("wkq", (C, 2 * H), F16, kind="ExternalInput")
    wv = nc.dram_tensor("wv", (C, H), F16, kind="ExternalInput")
    out = nc.dram_tensor("out", (T, H), F32, kind="ExternalOutput")
    outv = out.rearrange("(i p) h -> p i h", p=P)   # (128, 16, 64) view

    with ExitStack() as ctx:
        tc = ctx.enter_context(tile.TileContext(nc))
        const = ctx.enter_context(tc.tile_pool(name="const", bufs=1))
        ptp = ctx.enter_context(tc.tile_pool(name="ptp", bufs=2))
        sml = ctx.enter_context(tc.tile_pool(name="sml", bufs=2))
        psS = ctx.enter_context(tc.tile_pool(name="psS", bufs=2, space="PSUM"))
        psP = ctx.enter_context(tc.tile_pool(name="psP", bufs=2, space="PSUM"))
        psO = ctx.enter_context(tc.tile_pool(name="psO", bufs=1, space="PSUM"))
        psT = ctx.enter_context(tc.tile_pool(name="psT", bufs=1, space="PSUM"))

        # ---- persistent SBUF ----
        xt_sb = const.tile([P, CT, T], F16)        # x^T, c-tiled
        wkq_sb = const.tile([P, CT, 2 * H], F16)
        wv_sb = const.tile([P, CT, H], F16)
        kqn = const.tile([P, T], F16)              # rows 0:64 k^T, 64:128 q^T
        qk2 = const.tile([P, T], F16)              # rows 0:64 q^T, 64:128 k^T
        vt = const.tile([P, 2, QB], F16)           # v^T col-packed pairs
        vsb = const.tile([P, KT, H + 1], F16)      # V' tiles (v | ones-col)
        outn = const.tile([P, KT, H], F32)         # natural out tiles
        ident = const.tile([P, P], F16)
        wrm = const.tile([P, QB], F16)             # warm-up operand
        dumm = const.tile([P, 1], F16)             # ACT table preload target

        # ---- input DMAs first: weights, then x chunk-major on both rings
        xtv = xt.rearrange("(c p) t -> p c t", p=P)
        nc.sync.dma_start(wkq_sb[:], wkq.rearrange("(c p) m -> p c m", p=P))
        nc.sync.dma_start(wv_sb[:], wv.rearrange("(c p) m -> p c m", p=P))
        for g in range(NBLK):
            sl = ds(g * QB, QB)
            nc.sync.dma_start(xt_sb[:, 0:4, sl], xtv[:, 0:4, sl])
            nc.scalar.dma_start(xt_sb[:, 4:8, sl], xtv[:, 4:8, sl])

        # ---- constants (no DMA deps -> issue immediately) ----
        nc.gpsimd.memset(wrm[:], 0.25)
        nc.gpsimd.memset(vsb[:, :, H:H + 1], 1.0)  # V' ones-column
        make_identity(nc, ident)

        # ACT exp table preload (one-time ~2.7us) while x streams
        nc.scalar.activation(dumm[:], wrm[:, 0:1],
                             mybir.ActivationFunctionType.Exp, scale=1.0)

        # ---- PE warm-up while x loads: un-throttle the HAM clock gate
        for _ in range(N_WARM):
            pw = psP.tile([P, QB], F32, tag="p")
            nc.tensor.matmul(pw[:], wrm[:, 0:P], wrm[:], start=True, stop=True)

        def proj_kq(g):
            # kqn[:, g*QB:] = [Wk|Wq]^T x_g^T ; build qk2 dup halves on DVE
            sl = ds(g * QB, QB)
            pk = psP.tile([P, QB], F32, tag="p")
            for c in range(CT):
                nc.tensor.matmul(pk[:], wkq_sb[:, c, :], xt_sb[:, c, sl],
                                 start=(c == 0), stop=(c == CT - 1))
            nc.vector.tensor_copy(kqn[:, sl], pk[:])
            nc.vector.tensor_copy(qk2[0:H, sl], kqn[H:P, sl])
            nc.vector.tensor_copy(qk2[H:P, sl], kqn[0:H, sl])

        def proj_v(m):
            # col-packed pair: v^T for chunks 2m (rows 0:64), 2m+1 (64:128).
            # Separate PSUM banks per half: start=True clears has_written for
            # the whole bank, so interleaved groups cannot share one.
            pv0 = psP.tile([P, QB], F32, tag="p")
            pv1 = psP.tile([P, QB], F32, tag="p")
            for c in range(CT):
                nc.tensor.matmul(pv0[0:H, :], wv_sb[:, c, :],
                                 xt_sb[:, c, ds(2 * m * QB, QB)],
                                 start=(c == 0), stop=(c == CT - 1))
                nc.tensor.matmul(pv1[H:P, :], wv_sb[:, c, :],
                                 xt_sb[:, c, ds((2 * m + 1) * QB, QB)],
                                 start=(c == 0), stop=(c == CT - 1))
            nc.vector.tensor_copy(vt[0:H, m, :], pv0[0:H, :])
            nc.vector.tensor_copy(vt[H:P, m, :], pv1[H:P, :])

        def vnat(g):
            # transpose chunk g's v^T tiles to natural vsb tiles 4g..4g+3
            m, r = g // 2, g % 2
            pn = psT.tile([P, 4, H], F32, tag="t")
            for i in range(4):
                nc.tensor.matmul(pn[:, i, :],
                                 vt[ds(H * r, H), m, ds(P * i, P)],
                                 ident[ds(H * r, H), ds(H * r, H)],
                                 start=True, stop=True)
            nc.vector.tensor_copy(vsb[:, ds(4 * g, 4), 0:H], pn[:])

        def c0_of(b, j):
            return max(0, P * j - QB * b)

        def s_pair(b, pi):
            # scores for tiles (2pi, 2pi+1) of block b: row-packed K=64 pair
            # + ONE exp ACT call + diagonal masks. Returns pt for the PV.
            # Full-width score matmuls even on diagonal tiles: the dead region
            # is masked after exp, and a fully-written PSUM group lets one ACT
            # call cover the pair.
            ss = psS.tile([P, 2, QB], F32, tag="s")
            pt = ptp.tile([P, 2, QB], F16, tag="pt")
            qsl = ds(b * QB, QB)
            for r in (0, 1):
                j = 2 * pi + r
                ksl = ds(j * P, P)
                if r == 0:
                    nc.tensor.matmul(ss[:, 0, :], kqn[0:H, ksl],
                                     qk2[0:H, qsl], start=True, stop=True)
                else:
                    nc.tensor.matmul(ss[:, 1, :], qk2[H:P, ksl],
                                     kqn[H:P, qsl], start=True, stop=True)
            nc.scalar.activation(pt[:], ss[:],
                                 mybir.ActivationFunctionType.Exp, scale=SCALE)
            for r in (0, 1):
                j = 2 * pi + r
                if P * j >= QB * b:  # diagonal: zero keys s with s > query u
                    nc.gpsimd.affine_select(
                        out=pt[:, r, :], in_=pt[:, r, :],
                        compare_op=mybir.AluOpType.is_ge, fill=0.0,
                        base=-c0_of(b, j), pattern=[[1, QB]],
                        channel_multiplier=-1,
                    )
            return pt

        def pv_pair(b, pi, pt, po):
            jmax = 4 * b + 3
            for r in (0, 1):
                j = 2 * pi + r
                c0 = c0_of(b, j)
                nc.tensor.matmul(po[:, c0:], vsb[:, j, :], pt[:, r, c0:],
                                 start=(j == 0), stop=(j == jmax))

        def epilogue(b, po):
            posb = sml.tile([H + 1, QB], F16, tag="os")
            nc.vector.tensor_copy(posb[:], po[:])
            pn = psT.tile([P, 4, H + 1], F32, tag="t")
            for i in range(4):
                nc.tensor.matmul(pn[:, i, :], posb[:, ds(i * P, P)],
                                 ident[0:H + 1, 0:H + 1], start=True, stop=True)
            onat = sml.tile([P, 4, H + 1], F32, tag="on")
            nc.vector.tensor_copy(onat[:], pn[:])
            rc = sml.tile([P, 4, 1], F32, tag="rc")
            nc.vector.reciprocal(rc[:], onat[:, :, H:H + 1])
            nc.vector.tensor_tensor(outn[:, ds(4 * b, 4), :],
                                    onat[:, :, 0:H],
                                    rc[:].to_broadcast((P, 4, H)),
                                    mybir.AluOpType.mult)
            nc.sync.dma_start(outv[:, ds(4 * b, 4), :], outn[:, ds(4 * b, 4), :])

        # ---- hand-interleaved schedule ----
        proj_kq(0)
        proj_kq(1)
        proj_v(0)
        vnat(0)
        vnat(1)

        po0 = psO.tile([H + 1, QB], F32, tag="o")
        pt = s_pair(0, 0)
        pt_n = s_pair(0, 1)
        pv_pair(0, 0, pt, po0)
        proj_kq(2)
        pv_pair(0, 1, pt_n, po0)

        po1 = psO.tile([H + 1, QB], F32, tag="o")
        pt = s_pair(1, 0)
        epilogue(0, po0)
        pt_n = s_pair(1, 1)
        pv_pair(1, 0, pt, po1)
        pt = s_pair(1, 2)
        pv_pair(1, 1, pt_n, po1)
        proj_kq(3)
        pt_n = s_pair(1, 3)
        pv_pair(1, 2, pt, po1)
        proj_v(1)
        pv_pair(1, 3, pt_n, po1)

        po2 = psO.tile([H + 1, QB], F32, tag="o")
        pt = s_pair(2, 0)
        epilogue(1, po1)
        vnat(2)
        vnat(3)
        pt_n = s_pair(2, 1)
        pv_pair(2, 0, pt, po2)
        pt = s_pair(2, 2)
        pv_pair(2, 1, pt_n, po2)
        pt_n = s_pair(2, 3)
        pv_pair(2, 2, pt, po2)
        pt = s_pair(2, 4)
        pv_pair(2, 3, pt_n, po2)
        pt_n = s_pair(2, 5)
        pv_pair(2, 4, pt, po2)
        pv_pair(2, 5, pt_n, po2)

        po3 = psO.tile([H + 1, QB], F32, tag="o")
        pt = s_pair(3, 0)
        epilogue(2, po2)
        pt_n = s_pair(3, 1)
        pv_pair(3, 0, pt, po3)
        pt = s_pair(3, 2)
        pv_pair(3, 1, pt_n, po3)
        pt_n = s_pair(3, 3)
        pv_pair(3, 2, pt, po3)
        pt = s_pair(3, 4)
        pv_pair(3, 3, pt_n, po3)
        pt_n = s_pair(3, 5)
        pv_pair(3, 4, pt, po3)
        pt = s_pair(3, 6)
        pv_pair(3, 5, pt_n, po3)
        pt_n = s_pair(3, 7)
        pv_pair(3, 6, pt, po3)
        pv_pair(3, 7, pt_n, po3)
        epilogue(3, po3)

    nc.compile()
    return nc


_NC = None
LAST_EXEC_TIME_NS = None  # filled when BASS_TRACE=1 (read by test.py)
LAST_RESULT = None


def _get_nc():
    global _NC
    if _NC is None:
        _NC = build_bass()
    return _NC


def kernel(x, Wk, Wq, Wv):
    global LAST_EXEC_TIME_NS, LAST_RESULT
    x = np.ascontiguousarray(x, dtype=np.float16)
    wkq = np.ascontiguousarray(np.concatenate([Wk, Wq], axis=1), dtype=np.float16)
    wv = np.ascontiguousarray(Wv, dtype=np.float16)

    in_maps = []
    for b in range(B):
        in_maps.append({
            "xt": np.ascontiguousarray(x[b].T),
            "wkq": wkq,
            "wv": wv,
        })

    nc = _get_nc()
    res = run_bass_kernel_spmd(nc, in_maps, list(range(B)))
    LAST_EXEC_TIME_NS = res.exec_time_ns
    LAST_RESULT = res
    out = np.stack([np.ascontiguousarray(m["out"]) for m in res.results])
    return out.astype(np.float32)


# revision 5
# speedup vs baseline: 1.1871x; 1.0171x over previous
"""Trainium2 Bass kernel: single-head causal self-attention.

Problem: x:(8,2048,1024) f32, Wk/Wq/Wv:(1024,64) f32
  k,q,v = x@Wk, x@Wq, x@Wv ; S = q k^T / sqrt(64) causal-masked
  out = softmax(S) @ v  -> (8,2048,64) f32

Sharding: data-parallel over batch B=8 across the 8 NeuronCores (one batch
element per core).

Per-core design (fp16 matmul paths, fp32 PSUM accumulation):
  - x^T (C,T) fp16 streamed in 256KB pieces, chunk-major, across THREE DMA
    paths (sync HWDGE, scalar HWDGE, gpsimd SWDGE) so early chunks land
    first and projections start ~9-10us in. Weights lead each ring.
  - Warm-up matmuls un-throttle the PE HAM clock gate while x streams, and
    chain directly into the projections so the PE never re-throttles.
  - Projections: [Wk|Wq] packed (M=128) -> kqn (k rows 0:64, q rows 64:128);
    DVE cross-base copies build qk2 (q top, k bottom) so scores can be
    ROW-PACKED: two K=64 score matmuls run concurrently in PE halves.
  - V projection: col-packed pair for chunks (0,1); solo for chunks 2,3
    (their data arrives late; packing would stall the block-2 PV chain).
  - Scores per (block, tile-pair): 2 concurrent K=64 matmuls -> one fp32
    PSUM group (2 banks, double-buffered); ONE exp ACT call per pair.
  - Diagonal pairs: DVE multiply with static 0/1 mask tiles (built once at
    startup) zeroes the causally-dead region AFTER exp, off the gpsimd
    critical path.
  - PV: po(65,512) += V'_j^T P_j^T accumulated over tiles; row 64 = sum(P)
    (ones-column denominator trick).
  - Epilogue per block: PE-transpose to natural, DVE reciprocal+broadcast
    multiply, natural (2048,64) f32 output, per-block DMA on SP ring.
"""

import sys
from contextlib import ExitStack

import numpy as np

if "/opt/trn_rl_repo" not in sys.path:
    sys.path.insert(0, "/opt/trn_rl_repo")

import concourse.bacc as bacc
import concourse.mybir as mybir
import concourse.tile as tile
from concourse.bass import ds
from concourse.bass_utils import run_bass_kernel_spmd
from concourse.masks import make_identity

F32 = mybir.dt.float32
F16 = mybir.dt.float16

B, T, C, H = 8, 2048, 1024, 64
P = 128           # partitions
CT = C // P       # 8 c-tiles
NBLK = 4          # query blocks of 512
QB = T // NBLK    # 512 queries per block
KT = T // P       # 16 key tiles
SCALE = H ** -0.5
N_WARM = 10


def build_bass():
    nc = bacc.Bacc("TRN2")

    xt = nc.dram_tensor("xt", (C, T), F16, kind="ExternalInput")
    wkq = nc.dram_tensor("wkq", (C, 2 * H), F16, kind="ExternalInput")
    wv = nc.dram_tensor("wv", (C, H), F16, kind="ExternalInput")
    out = nc.dram_tensor("out", (T, H), F32, kind="ExternalOutput")
    outv = out.rearrange("(i p) h -> p i h", p=P)   # (128, 16, 64) view

    with ExitStack() as ctx:
        tc = ctx.enter_context(tile.TileContext(nc))
        const = ctx.enter_context(tc.tile_pool(name="const", bufs=1))
        ptp = ctx.enter_context(tc.tile_pool(name="ptp", bufs=2))
        sml = ctx.enter_context(tc.tile_pool(name="sml", bufs=2))
        psS = ctx.enter_context(tc.tile_pool(name="psS", bufs=2, space="PSUM"))
        psP = ctx.enter_context(tc.tile_pool(name="psP", bufs=2, space="PSUM"))
        psO = ctx.enter_context(tc.tile_pool(name="psO", bufs=1, space="PSUM"))
        psT = ctx.enter_context(tc.tile_pool(name="psT", bufs=1, space="PSUM"))

        # ---- persistent SBUF ----
        xt_sb = const.tile([P, CT, T], F16)        # x^T, c-tiled
        wkq_sb = const.tile([P, CT, 2 * H], F16)
        wv_sb = const.tile([P, CT, H], F16)
        kqn = const.tile([P, T], F16)              # rows 0:64 k^T, 64:128 q^T
        qk2 = const.tile([P, T], F16)              # rows 0:64 q^T, 64:128 k^T
        vt = const.tile([P, 2, QB], F16)           # v^T pair / solo staging
        vsb = const.tile([P, KT, H + 1], F16)      # V' tiles (v | ones-col)
        outn = const.tile([P, KT, H], F32)         # natural out tiles
        ident = const.tile([P, P], F16)
        wrm = const.tile([P, QB], F16)             # warm-up operand
        dumm = const.tile([P, 1], F16)             # ACT table preload target
        dmask = const.tile([P, 4, QB], F16)        # static diagonal masks

        # ---- input DMAs: 256KB pieces, chunk-major, 3 paths ----
        xtv = xt.rearrange("(c p) t -> p c t", p=P)

        def xdma(eng, g, clo):
            sl = ds(g * QB, QB)
            eng.dma_start(xt_sb[:, clo:clo + 2, sl], xtv[:, clo:clo + 2, sl])

        # sync HWDGE: c-tiles 0-3 of every chunk, chunk-major
        for g in range(NBLK):
            xdma(nc.sync, g, 0)
            xdma(nc.sync, g, 2)
        # scalar HWDGE: wkq first (gates proj0), then chunk0's other half
        nc.scalar.dma_start(wkq_sb[:], wkq.rearrange("(c p) m -> p c m", p=P))
        xdma(nc.scalar, 0, 4)
        xdma(nc.scalar, 0, 6)
        # gpsimd SWDGE: constants first (warm-up + masks), then wv + rest of x
        nc.gpsimd.memset(wrm[:], 0.25)
        nc.gpsimd.memset(vsb[:, :, H:H + 1], 1.0)  # V' ones-column
        make_identity(nc, ident)
        nc.gpsimd.memset(dmask[:], 1.0)
        for i in range(4):
            # mask_i[p, col] = 1 if col >= 128*i + p else 0
            nc.gpsimd.affine_select(
                out=dmask[:, i, :], in_=dmask[:, i, :],
                compare_op=mybir.AluOpType.is_ge, fill=0.0,
                base=-P * i, pattern=[[1, QB]], channel_multiplier=-1,
            )
        nc.gpsimd.dma_start(wv_sb[:], wv.rearrange("(c p) m -> p c m", p=P))
        for g in range(1, NBLK):
            xdma(nc.gpsimd, g, 4)
            xdma(nc.gpsimd, g, 6)

        # ACT exp table preload (one-time ~2.7us) while x streams
        nc.scalar.activation(dumm[:], wrm[:, 0:1],
                             mybir.ActivationFunctionType.Exp, scale=1.0)

        # ---- PE warm-up while x loads: un-throttle the HAM clock gate
        for _ in range(N_WARM):
            pw = psP.tile([P, QB], F32, tag="p")
            nc.tensor.matmul(pw[:], wrm[:, 0:P], wrm[:], start=True, stop=True)

        def proj_kq(g):
            # kqn[:, g*QB:] = [Wk|Wq]^T x_g^T ; build qk2 dup halves on DVE
            sl = ds(g * QB, QB)
            pk = psP.tile([P, QB], F32, tag="p")
            for c in range(CT):
                nc.tensor.matmul(pk[:], wkq_sb[:, c, :], xt_sb[:, c, sl],
                                 start=(c == 0), stop=(c == CT - 1))
            nc.vector.tensor_copy(kqn[:, sl], pk[:])
            nc.vector.tensor_copy(qk2[0:H, sl], kqn[H:P, sl])
            nc.vector.tensor_copy(qk2[H:P, sl], kqn[0:H, sl])

        def proj_v01():
            # col-packed pair: v^T chunk 0 (rows 0:64), chunk 1 (64:128).
            # Separate PSUM banks per half: start=True clears has_written for
            # the whole bank, so interleaved groups cannot share one.
            pv0 = psP.tile([P, QB], F32, tag="p")
            pv1 = psP.tile([P, QB], F32, tag="p")
            for c in range(CT):
                nc.tensor.matmul(pv0[0:H, :], wv_sb[:, c, :],
                                 xt_sb[:, c, ds(0, QB)],
                                 start=(c == 0), stop=(c == CT - 1))
                nc.tensor.matmul(pv1[H:P, :], wv_sb[:, c, :],
                                 xt_sb[:, c, ds(QB, QB)],
                                 start=(c == 0), stop=(c == CT - 1))
            nc.vector.tensor_copy(vt[0:H, 0, :], pv0[0:H, :])
            nc.vector.tensor_copy(vt[H:P, 0, :], pv1[H:P, :])

        def proj_v_solo(g):
            # solo M=64 v^T for late chunk g (2 or 3); row half r=g%2 of vt[1]
            r = g % 2
            pv = psP.tile([P, QB], F32, tag="p")
            for c in range(CT):
                nc.tensor.matmul(pv[ds(H * r, H), :], wv_sb[:, c, :],
                                 xt_sb[:, c, ds(g * QB, QB)],
                                 start=(c == 0), stop=(c == CT - 1))
            nc.vector.tensor_copy(vt[ds(H * r, H), 1, :], pv[ds(H * r, H), :])

        def vnat(g):
            # transpose chunk g's v^T tiles to natural vsb tiles 4g..4g+3
            m, r = g // 2, g % 2
            pn = psT.tile([P, 4, H], F32, tag="t")
            for i in range(4):
                nc.tensor.matmul(pn[:, i, :],
                                 vt[ds(H * r, H), m, ds(P * i, P)],
                                 ident[ds(H * r, H), ds(H * r, H)],
                                 start=True, stop=True)
            nc.vector.tensor_copy(vsb[:, ds(4 * g, 4), 0:H], pn[:])

        def c0_of(b, j):
            return max(0, P * j - QB * b)

        def s_pair(b, pi):
            # scores for tiles (2pi, 2pi+1) of block b: row-packed K=64 pair
            # + ONE exp ACT call + DVE diagonal masks. Returns pt for the PV.
            ss = psS.tile([P, 2, QB], F32, tag="s")
            pt = ptp.tile([P, 2, QB], F16, tag="pt")
            qsl = ds(b * QB, QB)
            for r in (0, 1):
                j = 2 * pi + r
                ksl = ds(j * P, P)
                if r == 0:
                    nc.tensor.matmul(ss[:, 0, :], kqn[0:H, ksl],
                                     qk2[0:H, qsl], start=True, stop=True)
                else:
                    nc.tensor.matmul(ss[:, 1, :], qk2[H:P, ksl],
                                     kqn[H:P, qsl], start=True, stop=True)
            # ACT can skip columns left of the pair's first alive column
            ca = min(c0_of(b, 2 * pi), c0_of(b, 2 * pi + 1))
            ca = (ca // P) * P
            nc.scalar.activation(pt[:, :, ca:], ss[:, :, ca:],
                                 mybir.ActivationFunctionType.Exp, scale=SCALE)
            for r in (0, 1):
                j = 2 * pi + r
                if P * j >= QB * b:  # diagonal: zero keys s with s > query u
                    i = j - 4 * b
                    c0 = c0_of(b, j)
                    nc.vector.tensor_tensor(pt[:, r, c0:], pt[:, r, c0:],
                                            dmask[:, i, c0:],
                                            mybir.AluOpType.mult)
            return pt

        def pv_pair(b, pi, pt, po):
            jmax = 4 * b + 3
            for r in (0, 1):
                j = 2 * pi + r
                c0 = c0_of(b, j)
                nc.tensor.matmul(po[:, c0:], vsb[:, j, :], pt[:, r, c0:],
                                 start=(j == 0), stop=(j == jmax))

        def epilogue(b, po):
            posb = sml.tile([H + 1, QB], F16, tag="os")
            nc.vector.tensor_copy(posb[:], po[:])
            pn = psT.tile([P, 4, H + 1], F32, tag="t")
            for i in range(4):
                nc.tensor.matmul(pn[:, i, :], posb[:, ds(i * P, P)],
                                 ident[0:H + 1, 0:H + 1], start=True, stop=True)
            onat = sml.tile([P, 4, H + 1], F32, tag="on")
            nc.vector.tensor_copy(onat[:], pn[:])
            rc = sml.tile([P, 4, 1], F32, tag="rc")
            nc.vector.reciprocal(rc[:], onat[:, :, H:H + 1])
            nc.vector.tensor_tensor(outn[:, ds(4 * b, 4), :],
                                    onat[:, :, 0:H],
                                    rc[:].to_broadcast((P, 4, H)),
                                    mybir.AluOpType.mult)
            nc.sync.dma_start(outv[:, ds(4 * b, 4), :], outn[:, ds(4 * b, 4), :])

        # ---- hand-interleaved schedule ----
        proj_kq(0)
        proj_kq(1)
        proj_v01()
        vnat(0)
        vnat(1)

        po0 = psO.tile([H + 1, QB], F32, tag="o")
        pt = s_pair(0, 0)
        pt_n = s_pair(0, 1)
        pv_pair(0, 0, pt, po0)
        proj_kq(2)
        pv_pair(0, 1, pt_n, po0)

        po1 = psO.tile([H + 1, QB], F32, tag="o")
        pt = s_pair(1, 0)
        epilogue(0, po0)
        pt_n = s_pair(1, 1)
        pv_pair(1, 0, pt, po1)
        pt = s_pair(1, 2)
        pv_pair(1, 1, pt_n, po1)
        proj_v_solo(2)
        pt_n = s_pair(1, 3)
        pv_pair(1, 2, pt, po1)
        vnat(2)
        pv_pair(1, 3, pt_n, po1)

        po2 = psO.tile([H + 1, QB], F32, tag="o")
        pt = s_pair(2, 0)
        epilogue(1, po1)
        pt_n = s_pair(2, 1)
        pv_pair(2, 0, pt, po2)
        proj_kq(3)
        pt = s_pair(2, 2)
        pv_pair(2, 1, pt_n, po2)
        pt_n = s_pair(2, 3)
        pv_pair(2, 2, pt, po2)
        proj_v_solo(3)
        pt = s_pair(2, 4)
        pv_pair(2, 3, pt_n, po2)
        vnat(3)
        pt_n = s_pair(2, 5)
        pv_pair(2, 4, pt, po2)
        pv_pair(2, 5, pt_n, po2)

        po3 = psO.tile([H + 1, QB], F32, tag="o")
        pt = s_pair(3, 0)
        epilogue(2, po2)
        pt_n = s_pair(3, 1)
        pv_pair(3, 0, pt, po3)
        pt = s_pair(3, 2)
        pv_pair(3, 1, pt_n, po3)
        pt_n = s_pair(3, 3)
        pv_pair(3, 2, pt, po3)
        pt = s_pair(3, 4)
        pv_pair(3, 3, pt_n, po3)
        pt_n = s_pair(3, 5)
        pv_pair(3, 4, pt, po3)
        pt = s_pair(3, 6)
        pv_pair(3, 5, pt_n, po3)
        pt_n = s_pair(3, 7)
        pv_pair(3, 6, pt, po3)
        pv_pair(3, 7, pt_n, po3)
        epilogue(3, po3)

    nc.compile()
    return nc


_NC = None
LAST_EXEC_TIME_NS = None  # filled when BASS_TRACE=1 (read by test.py)
LAST_RESULT = None


def _get_nc():
    global _NC
    if _NC is None:
        _NC = build_bass()
    return _NC


def kernel(x, Wk, Wq, Wv):
    global LAST_EXEC_TIME_NS, LAST_RESULT
    x = np.ascontiguousarray(x, dtype=np.float16)
    wkq = np.ascontiguousarray(np.concatenate([Wk, Wq], axis=1), dtype=np.float16)
    wv = np.ascontiguousarray(Wv, dtype=np.float16)

    in_maps = []
    for b in range(B):
        in_maps.append({
            "xt": np.ascontiguousarray(x[b].T),
            "wkq": wkq,
            "wv": wv,
        })

    nc = _get_nc()
    res = run_bass_kernel_spmd(nc, in_maps, list(range(B)))
    LAST_EXEC_TIME_NS = res.exec_time_ns
    LAST_RESULT = res
    out = np.stack([np.ascontiguousarray(m["out"]) for m in res.results])
    return out.astype(np.float32)
